# revision 9
# baseline (speedup 1.0000x reference)
"""Mixtral sparse-MoE block with per-expert LoRA adapters on 8 Trainium2 cores.

Problem shapes: B=2, S=1024, H=2048, F=7168, E=8, R=32, top-K=2.
T = B*S = 2048 tokens.

Sharding: tensor-parallel over the FFN dim F. Core c owns rows
[c*896:(c+1)*896] of W1/W3 (and the matching B1/B3 LoRA rows) and the same
columns of W2/A2. Gating (softmax + top-2) runs on the host; the per-slot
expert selection is shipped as a one-hot [E*R, T] mask so the LoRA paths are
dense matmuls over a 256-wide contraction (no gathers on device).

Everything after the silu is linear in x2 = silu(x1)*x3*rw, so each core
computes an exact partial [H, T] output over its F-shard and the host sums
the 8 partials.

Layout is feature-major ([feature, token]) end to end, which makes every
matmul natural (contraction on partitions) with zero on-device transposes:
  base1[f,t]   = sum_h W1[f,h] x[t,h]          lhsT=W1T chunk, rhs=xT
  a1T[er,t]    = sum_h A1flat[er,h] x[t,h]     lhsT=A1flatT,   rhs=xT
  lora1[f,t]  += sum_er B1flat[f,er] m1[er,t]  lhsT=B1flat.T,  rhs=m1T
  x1 = base1 + lora1 via PSUM accumulation (identity-matmul re-injects base)
  down[h,t]   += sum_f W2[h,f] x2s[f,t]        lhsT=W2T chunk, rhs=x2s
  a2T[er,t]    = sum_f A2flat[er,f] x2s[f,t]   lhsT=A2flatT,   rhs=x2s
  lora2[h,t]  += sum_er B2flat[h,er] m2s[er,t] lhsT=B2flat.T,  rhs=m2sT

All matmul operands are bf16 (fp32 matmul is 4x slower on TRN2); PSUM
accumulation is fp32.
"""

import os
import sys
from contextlib import ExitStack

import numpy as np

try:
    import concourse.bass as bass  # noqa: F401
except ImportError:
    sys.path.insert(0, "/opt/trn_rl_repo")

import ml_dtypes

import concourse.bass as bass
import concourse.mybir as mybir
import concourse.tile as tile
from concourse import bacc
from concourse.bass_utils import run_bass_kernel_spmd

BF16 = mybir.dt.bfloat16
F32 = mybir.dt.float32
NPBF16 = ml_dtypes.bfloat16

B, S, H, F, E, R, K = 2, 1024, 2048, 7168, 8, 32, 2
T = B * S                      # 2048 tokens
ER = E * R                     # 256
NCORES = 8
FS = F // NCORES               # 896 per-core F shard
NH = H // 128                  # 16 h-chunks
NF = FS // 128                 # 7 f-chunks (per core)
NER = ER // 128                # 2 er-chunks
TBLK = 512
NT = T // TBLK                 # 4 token blocks


def build_nc():
    nc = bacc.Bacc(None)

    xT = nc.declare_dram_parameter("xT", [NH, 128, T], BF16, isOutput=False)
    w1t = nc.declare_dram_parameter("w1t", [NH, 128, FS], BF16, isOutput=False)
    w3t = nc.declare_dram_parameter("w3t", [NH, 128, FS], BF16, isOutput=False)
    w2t = nc.declare_dram_parameter("w2t", [NF, 128, H], BF16, isOutput=False)
    a1t = nc.declare_dram_parameter("a1t", [NH, 128, ER], BF16, isOutput=False)
    a3t = nc.declare_dram_parameter("a3t", [NH, 128, ER], BF16, isOutput=False)
    b1t = nc.declare_dram_parameter("b1t", [NER, 128, FS], BF16, isOutput=False)
    b3t = nc.declare_dram_parameter("b3t", [NER, 128, FS], BF16, isOutput=False)
    a2t = nc.declare_dram_parameter("a2t", [NF, 128, ER], BF16, isOutput=False)
    b2t = nc.declare_dram_parameter("b2t", [NER, 128, H], BF16, isOutput=False)
    maskt = nc.declare_dram_parameter("maskt", [K, NER, 128, T], BF16, isOutput=False)
    rwr = nc.declare_dram_parameter("rwr", [K, 1, T], F32, isOutput=False)
    ident = nc.declare_dram_parameter("ident", [128, 128], BF16, isOutput=False)
    outT = nc.declare_dram_parameter("outT", [NH, 128, T], F32, isOutput=True)

    with tile.TileContext(nc) as tc, ExitStack() as ctx:
        resw = ctx.enter_context(tc.tile_pool(name="resw", bufs=1))
        xsp = ctx.enter_context(tc.tile_pool(name="xsp", bufs=1))
        actp = ctx.enter_context(tc.tile_pool(name="actp", bufs=1))
        mp_ = ctx.enter_context(tc.tile_pool(name="mp", bufs=1))
        trans = ctx.enter_context(tc.tile_pool(name="trans", bufs=4))
        outp = ctx.enter_context(tc.tile_pool(name="outp", bufs=2))
        psp = ctx.enter_context(tc.tile_pool(name="psp", bufs=2, space="PSUM"))

        # ---- per-block input streamers ----
        def load_block_inputs(tb):
            tsl = slice(tb * TBLK, (tb + 1) * TBLK)
            xs = []
            for h in range(NH):
                xt_ = xsp.tile([128, TBLK], BF16, name=f"x{h}", tag=f"x{h}")
                nc.sync.dma_start(out=xt_, in_=xT[h][:, tsl])
                xs.append(xt_)
            msk = [[None] * NER for _ in range(K)]
            for k in range(K):
                for er in range(NER):
                    m_ = mp_.tile([128, TBLK], BF16, name=f"mk{k}_{er}",
                                  tag=f"mk{k}_{er}")
                    nc.sync.dma_start(out=m_, in_=maskt[k][er][:, tsl])
                    msk[k][er] = m_
            rws = []
            for k in range(K):
                r_ = mp_.tile([128, TBLK], F32, name=f"rw{k}", tag=f"rw{k}")
                nc.sync.dma_start(out=r_, in_=rwr[k][:, tsl].to_broadcast([128, TBLK]))
                rws.append(r_)
            return xs, msk, rws

        # ---- resident weights, emitted in first-use order so the HWDGE
        # queue feeds phase A of block 0 as early as possible ----
        def resident(src, n, shape, nm):
            ts = []
            for i in range(n):
                t_ = resw.tile(shape, BF16, name=f"{nm}{i}", tag=f"{nm}{i}")
                nc.sync.dma_start(out=t_, in_=src[i])
                ts.append(t_)
            return ts

        pre0 = load_block_inputs(0)
        w1s, w3s = [], []
        for h in range(NH):
            t1 = resw.tile([128, FS], BF16, name=f"w1s{h}", tag=f"w1s{h}")
            nc.sync.dma_start(out=t1, in_=w1t[h])
            w1s.append(t1)
            t3 = resw.tile([128, FS], BF16, name=f"w3s{h}", tag=f"w3s{h}")
            nc.sync.dma_start(out=t3, in_=w3t[h])
            w3s.append(t3)
        a1s = resident(a1t, NH, [128, ER], "a1s")
        a3s = resident(a3t, NH, [128, ER], "a3s")
        b1s = resident(b1t, NER, [128, FS], "b1s")
        b3s = resident(b3t, NER, [128, FS], "b3s")
        idn = resw.tile([128, 128], BF16, name="idn", tag="idn")
        nc.sync.dma_start(out=idn, in_=ident[:, :])
        w2s = resident(w2t, NF, [128, H], "w2s")
        a2s = resident(a2t, NF, [128, ER], "a2s")
        b2s = resident(b2t, NER, [128, H], "b2s")

        for tb in range(NT):
            tsl = slice(tb * TBLK, (tb + 1) * TBLK)
            xs, msk, rws = pre0 if tb == 0 else load_block_inputs(tb)

            # ---- phase A: shared base1/base3 and LoRA down-projections ----
            base1 = [None] * NF
            base3 = [None] * NF
            for f in range(NF):
                fsl = slice(f * 128, (f + 1) * 128)
                ps1 = psp.tile([128, TBLK], F32, name="ps1", tag="pA")
                ps3 = psp.tile([128, TBLK], F32, name="ps3", tag="pB")
                for h in range(NH):
                    nc.tensor.matmul(ps1, w1s[h][:, fsl], xs[h], start=(h == 0), stop=(h == NH - 1))
                    nc.tensor.matmul(ps3, w3s[h][:, fsl], xs[h], start=(h == 0), stop=(h == NH - 1))
                b1_ = actp.tile([128, TBLK], BF16, name=f"b1_{f}", tag=f"b1_{f}")
                nc.scalar.copy(b1_, ps1)
                base1[f] = b1_
                b3_ = actp.tile([128, TBLK], BF16, name=f"b3_{f}", tag=f"b3_{f}")
                nc.scalar.copy(b3_, ps3)
                base3[f] = b3_

            m1 = [[None] * NER for _ in range(K)]
            m3 = [[None] * NER for _ in range(K)]
            for er in range(NER):
                ers = slice(er * 128, (er + 1) * 128)
                psa1 = psp.tile([128, TBLK], F32, name="psa1", tag="pA")
                psa3 = psp.tile([128, TBLK], F32, name="psa3", tag="pB")
                for h in range(NH):
                    nc.tensor.matmul(psa1, a1s[h][:, ers], xs[h],
                                     start=(h == 0), stop=(h == NH - 1))
                    nc.tensor.matmul(psa3, a3s[h][:, ers], xs[h],
                                     start=(h == 0), stop=(h == NH - 1))
                for k in range(K):
                    m1_ = actp.tile([128, TBLK], BF16, name=f"m1_{k}{er}",
                                    tag=f"m1_{k}{er}")
                    nc.vector.tensor_mul(m1_, psa1, msk[k][er])
                    m1[k][er] = m1_
                    m3_ = actp.tile([128, TBLK], BF16, name=f"m3_{k}{er}",
                                    tag=f"m3_{k}{er}")
                    nc.vector.tensor_mul(m3_, psa3, msk[k][er])
                    m3[k][er] = m3_

            # ---- phase B: per-slot LoRA up-proj, silu, x2s; then a2 ----
            x2s = [[None] * NF for _ in range(K)]
            m2 = [[None] * NER for _ in range(K)]
            for k in range(K):
                for f in range(NF):
                    fsl = slice(f * 128, (f + 1) * 128)
                    psA = psp.tile([128, TBLK], F32, name="psA", tag="pA")
                    nc.tensor.matmul(psA, b1s[0][:, fsl], m1[k][0], start=True, stop=False)
                    nc.tensor.matmul(psA, b1s[1][:, fsl], m1[k][1], start=False, stop=False)
                    nc.tensor.matmul(psA, idn, base1[f], start=False, stop=True)
                    psB = psp.tile([128, TBLK], F32, name="psB", tag="pB")
                    nc.tensor.matmul(psB, b3s[0][:, fsl], m3[k][0], start=True, stop=False)
                    nc.tensor.matmul(psB, b3s[1][:, fsl], m3[k][1], start=False, stop=False)
                    nc.tensor.matmul(psB, idn, base3[f], start=False, stop=True)
                    sl_ = trans.tile([128, TBLK], BF16, name="sl", tag="sl")
                    nc.scalar.activation(sl_, psA, mybir.ActivationFunctionType.Silu)
                    x3s_ = trans.tile([128, TBLK], BF16, name="x3s", tag="x3s")
                    nc.vector.tensor_mul(x3s_, psB, rws[k])
                    x2_ = actp.tile([128, TBLK], BF16, name=f"x2_{k}{f}",
                                    tag=f"x2_{k}{f}")
                    nc.vector.tensor_mul(x2_, sl_, x3s_)
                    x2s[k][f] = x2_
                for er in range(NER):
                    ers = slice(er * 128, (er + 1) * 128)
                    psa2 = psp.tile([128, TBLK], F32, name="psa2", tag="pA")
                    for f in range(NF):
                        nc.tensor.matmul(psa2, a2s[f][:, ers], x2s[k][f],
                                         start=(f == 0), stop=(f == NF - 1))
                    m2_ = actp.tile([128, TBLK], BF16, name=f"m2_{k}{er}",
                                    tag=f"m2_{k}{er}")
                    nc.vector.tensor_mul(m2_, psa2, msk[k][er])
                    m2[k][er] = m2_

            # ---- phase C: down-proj + lora2, both slots into one PSUM ----
            for h in range(NH):
                hsl = slice(h * 128, (h + 1) * 128)
                psD = psp.tile([128, TBLK], F32, name="psD", tag="pD")
                first = True
                for k in range(K):
                    for f in range(NF):
                        nc.tensor.matmul(psD, w2s[f][:, hsl], x2s[k][f],
                                         start=first, stop=False)
                        first = False
                    for er in range(NER):
                        last = (k == K - 1) and (er == NER - 1)
                        nc.tensor.matmul(psD, b2s[er][:, hsl], m2[k][er],
                                         start=False, stop=last)
                o_ = outp.tile([128, TBLK], F32, name="osb", tag="osb")
                nc.scalar.copy(o_, psD)
                nc.sync.dma_start(out=outT[h][:, tsl], in_=o_)

    nc.finalize()
    return nc


def prepare_inputs(hidden_states, Wg, W1, W2, W3, A1, B1, A2, B2, A3, B3):
    """Host preprocessing: routing + per-core weight slicing/casting."""
    x = np.ascontiguousarray(hidden_states.reshape(T, H)).astype(np.float32)

    logits = x @ Wg.T.astype(np.float32)
    m = logits.max(-1, keepdims=True)
    p = np.exp(logits - m, dtype=np.float32)
    p /= p.sum(-1, keepdims=True)
    sel = np.argsort(-p, axis=-1, kind="stable")[:, :K]      # [T, K]
    rw = np.take_along_axis(p, sel, axis=1)
    rw = (rw / rw.sum(-1, keepdims=True)).astype(np.float32)  # [T, K]

    xT_np = np.ascontiguousarray(x.T).astype(NPBF16).reshape(NH, 128, T)

    # per-slot one-hot masks over the (e, r) axis, transposed to [ER, T]
    maskt_np = np.zeros((K, ER, T), dtype=NPBF16)
    for k in range(K):
        onehot = np.zeros((T, E), np.float32)
        onehot[np.arange(T), sel[:, k]] = 1.0
        maskt_np[k] = np.repeat(onehot, R, axis=1).T.astype(NPBF16)
    maskt_np = maskt_np.reshape(K, NER, 128, T)
    rwr_np = np.ascontiguousarray(rw.T).reshape(K, 1, T).astype(np.float32)
    ident_np = np.eye(128, dtype=NPBF16)

    # flattened LoRA tensors (full copies; small)
    A1f = A1.reshape(ER, H)                      # [er, H]
    A3f = A3.reshape(ER, H)
    B2f = B2.transpose(0, 2, 1).reshape(ER, H)   # [er, H]
    a1t_np = np.ascontiguousarray(A1f.T).astype(NPBF16).reshape(NH, 128, ER)
    a3t_np = np.ascontiguousarray(A3f.T).astype(NPBF16).reshape(NH, 128, ER)
    b2t_np = np.ascontiguousarray(B2f).astype(NPBF16).reshape(NER, 128, H)

    in_maps = []
    for c in range(NCORES):
        fs = slice(c * FS, (c + 1) * FS)
        w1T = np.ascontiguousarray(W1[fs].T).astype(NPBF16)   # [H, FS]
        w3T = np.ascontiguousarray(W3[fs].T).astype(NPBF16)
        w1t_np = w1T.reshape(NH, 128, FS)
        w3t_np = w3T.reshape(NH, 128, FS)
        w2T = np.ascontiguousarray(W2[:, fs].T).astype(NPBF16)  # [FS, H]
        w2t_np = w2T.reshape(NF, 128, H)
        b1f = B1[:, fs, :].transpose(0, 2, 1).reshape(ER, FS)   # [er, f]
        b3f = B3[:, fs, :].transpose(0, 2, 1).reshape(ER, FS)
        b1t_np = np.ascontiguousarray(b1f).astype(NPBF16).reshape(NER, 128, FS)
        b3t_np = np.ascontiguousarray(b3f).astype(NPBF16).reshape(NER, 128, FS)
        a2f = A2[:, :, fs].reshape(ER, FS)                      # [er, f]
        a2t_np = np.ascontiguousarray(a2f.T).astype(NPBF16).reshape(NF, 128, ER)

        in_maps.append({
            "xT": xT_np, "w1t": w1t_np, "w3t": w3t_np, "w2t": w2t_np,
            "a1t": a1t_np, "a3t": a3t_np, "b1t": b1t_np, "b3t": b3t_np,
            "a2t": a2t_np, "b2t": b2t_np, "maskt": maskt_np,
            "rwr": rwr_np, "ident": ident_np,
        })
    return in_maps


_CACHED_NC = None


def kernel(hidden_states, Wg, W1, W2, W3, A1, B1, A2, B2, A3, B3,
           _trace=False, _tmpdir=None):
    global _CACHED_NC
    in_maps = prepare_inputs(hidden_states, Wg, W1, W2, W3,
                             A1, B1, A2, B2, A3, B3)
    if _CACHED_NC is None:
        _CACHED_NC = build_nc()
    nc = _CACHED_NC
    res = run_bass_kernel_spmd(nc, in_maps, list(range(NCORES)),
                               trace=_trace, tmpdir=_tmpdir)
    acc = np.zeros((NH, 128, T), np.float64)
    for c in range(NCORES):
        acc += res.results[c]["outT"].astype(np.float64)
    out = acc.reshape(H, T).T.astype(np.float32).reshape(B, S, H)
    kernel.last_results = res
    return out


if __name__ == "__main__":
    nc = build_nc()
    print("built ok")
